# revision 13
# baseline (speedup 1.0000x reference)
"""Trainium2 Bass kernel for nn_MultiHeadAttention_47330539602717.

Math (per batch b, head h; q/k projections are dead code in the reference):
    vpT   = Wv^T @ v_b^T                        (1024, 4096)  [no bias]
    A^T_h = i_h @ vpT_h + (i_h @ bv_h)[:,None]  (128 q, 4096 s)
    P1    = exp(A^T) / colsum(exp(A^T))         softmax over q (partition dim)
    exp2  = exp(9 * P1)
    Pfold[l, qr] = sum_j exp2[l, qr + 128 j]    (torch raw .view fold)
    w[q]  = sum_l Pfold[l, q] / Z2[l],  Z2[l] = sum_qr Pfold[l, qr]
    x_h   = w @ i_h                             (64,)
    out_b = concat_h(x_h) @ Wo + bo             (1, 1024)

Sharding: data-parallel over batch. Core c handles batch b=c (all 16 heads).
Engine plan per core: PE does the big GEMM (f32r full-rate), per-head A^T,
S1 column-sums (indicator matmuls), and S1-replication; ACT does exp1 (with
the bv bias folded per-partition), exp2, and 1/S1 via Ln+Exp(-x); DVE does
the softmax1 divide-multiply, the j-fold adds, and PSUM->SBUF moves.
"""

import sys

import numpy as np

sys.path.insert(0, "/opt/trn_rl_repo")

from contextlib import ExitStack

import concourse.bacc as bacc
import concourse.tile as tile
from concourse import mybir
from concourse.bass_utils import run_bass_kernel_spmd

F32 = mybir.dt.float32
F32R = mybir.dt.float32r
BF16 = mybir.dt.bfloat16
EXP = mybir.ActivationFunctionType.Exp
LN = mybir.ActivationFunctionType.Ln
AX = mybir.AxisListType.X

B, LQ, S, D, H = 8, 128, 4096, 1024, 16
DK = D // H          # 64
KD = 8               # k blocks of 128 over D
SCP = 4              # outer s chunks (1024 cols each)
SUB = 2              # 512-col substeps per chunk
NCH = 512
SMOOTH = 9.0


def r(ap):
    return ap.bitcast(F32R)


def build_program():
    nc = bacc.Bacc("TRN2", target_bir_lowering=False, debug=False,
                   num_devices=8)

    vT_d = nc.dram_tensor("vT", [D, S], F32, kind="ExternalInput").ap()
    iT_d = nc.dram_tensor("iT", [128, 8, 128], F32, kind="ExternalInput").ap()
    iN_d = nc.dram_tensor("iN", [128, H, DK], BF16, kind="ExternalInput").ap()
    cb_d = nc.dram_tensor("cb", [128, H], F32, kind="ExternalInput").ap()
    Wv_d = nc.dram_tensor("Wv", [D, D], F32, kind="ExternalInput").ap()
    Wo_d = nc.dram_tensor("Wo", [D, D], F32, kind="ExternalInput").ap()
    bo_d = nc.dram_tensor("bo", [1, D], F32, kind="ExternalInput").ap()
    ones_d = nc.dram_tensor("ones", [1, 128], F32, kind="ExternalInput").ap()
    ind_d = nc.dram_tensor("ind", [128, 8, 8], F32, kind="ExternalInput").ap()
    out_d = nc.dram_tensor("out", [1, D], F32, kind="ExternalOutput").ap()

    with tile.TileContext(nc) as tc, ExitStack() as ctx:
        singles = ctx.enter_context(tc.tile_pool(name="singles", bufs=1))
        vstream = ctx.enter_context(tc.tile_pool(name="vstream", bufs=2))
        vppool = ctx.enter_context(tc.tile_pool(name="vppool", bufs=2))
        expap = ctx.enter_context(tc.tile_pool(name="expap", bufs=2))
        e2pool = ctx.enter_context(tc.tile_pool(name="e2pool", bufs=2))
        smallp = ctx.enter_context(tc.tile_pool(name="smalls", bufs=2))
        rowp = ctx.enter_context(tc.tile_pool(name="rowp", bufs=1))
        vp_ps = ctx.enter_context(tc.tile_pool(name="vp_ps", bufs=2, space="PSUM"))
        a_ps = ctx.enter_context(tc.tile_pool(name="a_ps", bufs=2, space="PSUM"))
        R_ps = ctx.enter_context(tc.tile_pool(name="R_ps", bufs=2, space="PSUM"))
        s1_ps = ctx.enter_context(tc.tile_pool(name="s1_ps", bufs=1, space="PSUM"))

        # ---- constants / weights ----
        Wv_sb = singles.tile([128, KD, D], F32)       # ktile k at [:, k, :]
        nc.sync.dma_start(out=r(Wv_sb), in_=r(Wv_d.rearrange("(k p) c -> p k c", p=128)))
        iT_sb = singles.tile([128, 8, 128], F32)      # head h: [64*(h%2):+64, h//2, :]
        nc.sync.dma_start(out=r(iT_sb), in_=r(iT_d))
        iN_sb = singles.tile([128, H, DK], BF16)      # i_h as (q, d) at [:, h, :]
        nc.sync.dma_start(out=iN_sb, in_=iN_d)
        cb_sb = singles.tile([128, H], F32)
        nc.sync.dma_start(out=cb_sb, in_=cb_d)
        bo_sb = singles.tile([1, D], F32)
        nc.sync.dma_start(out=r(bo_sb), in_=r(bo_d))
        ones_row = singles.tile([1, 128], F32)        # lhsT for replicate (K=1,M=128)
        nc.sync.dma_start(out=r(ones_row), in_=r(ones_d))
        ind8 = singles.tile([128, 8, 8], F32)         # indicator lhsT per head-in-half
        nc.sync.dma_start(out=r(ind8), in_=r(ind_d))
        Pfold = singles.tile([128, H, LQ], F32)       # per-head fold accumulators

        # ---- main loop over 1024-col chunks ----
        for scp in range(SCP):
            vpT = []  # two (128, KD, 512) SBUF tiles, one per 512 substep
            for sub in range(SUB):
                sidx = scp * SUB + sub
                vT_sb = vstream.tile([128, KD, NCH], F32, tag="vT")
                nc.sync.dma_start(
                    out=r(vT_sb),
                    in_=r(vT_d[:, sidx * NCH:(sidx + 1) * NCH].rearrange(
                        "(k p) s -> p k s", p=128)),
                )
                vp_sb = vppool.tile([128, KD, NCH], F32, tag="vp")
                for cb in range(KD):
                    vp_p = vp_ps.tile([128, NCH], F32, tag="vp_p")
                    for k in range(KD):
                        nc.tensor.matmul(
                            vp_p,
                            lhsT=r(Wv_sb[:, k, cb * 128:(cb + 1) * 128]),
                            rhs=r(vT_sb[:, k, :]),
                            start=(k == 0), stop=(k == KD - 1),
                        )
                    nc.vector.tensor_copy(r(vp_sb[:, cb, :]), r(vp_p))
                vpT.append(vp_sb)

            for hb in range(2):
                s1_p = s1_ps.tile([8, SUB, NCH], F32, tag="s1")
                expq = []  # two (128, 4, 1024) expA quarter tiles
                for qb in range(2):
                    expA = expap.tile([128, 4, SUB * NCH], F32, tag="expa")
                    expq.append(expA)
                    for hi4 in range(4):
                        h = hb * 8 + qb * 4 + hi4
                        hi8 = qb * 4 + hi4
                        po = 64 * (h % 2)
                        for sub in range(SUB):
                            a_p = a_ps.tile([128, NCH], F32, tag="a_p")
                            nc.tensor.matmul(
                                a_p,
                                lhsT=r(iT_sb[po:po + 64, h // 2, :]),
                                rhs=r(vpT[sub][po:po + 64, h // 2, :]),
                                start=True, stop=True,
                            )
                            nc.scalar.activation(
                                r(expA[:, hi4, sub * NCH:(sub + 1) * NCH]),
                                a_p, EXP, bias=cb_sb[:, h:h + 1])
                        for sub in range(SUB):
                            nc.tensor.matmul(
                                s1_p[:, sub, :],
                                lhsT=r(ind8[:, hi8, :]),
                                rhs=r(expA[:, hi4, sub * NCH:(sub + 1) * NCH]),
                                start=(hi8 == 0), stop=(hi8 == 7),
                            )
                # 1/S1 for the half: spread the (8,1024) rows over all 128
                # partitions via DMA, reciprocal on DVE, then flatten to a
                # partition-0 row for the PE replicate matmuls
                s1_sb = smallp.tile([8, SUB * NCH], F32, tag="s1sb")
                nc.scalar.activation(s1_sb, s1_p,
                                     mybir.ActivationFunctionType.Copy)
                s1_sq = smallp.tile([128, 64], F32, tag="s1sq")
                nc.sync.dma_start(out=s1_sq, in_=s1_sb)
                rsq = smallp.tile([128, 64], F32, tag="rsq")
                nc.vector.reciprocal(rsq, s1_sq)
                for qb in range(2):
                    expA = expq[qb]
                    # PE matmul operands must share base partition 0: flatten
                    # this quarter's 1/S1 rows into a partition-0 row tile
                    rS1r = rowp.tile([1, 4, SUB * NCH], F32, tag="rS1r")
                    nc.sync.dma_start(
                        out=r(rS1r), in_=r(rsq[qb * 64:(qb + 1) * 64, :]))
                    exp2 = e2pool.tile([128, 4, SUB * NCH], BF16, tag="exp2")
                    for hi4 in range(4):
                        h = hb * 8 + qb * 4 + hi4
                        hi8 = qb * 4 + hi4
                        for sub in range(SUB):
                            R_p = R_ps.tile([128, NCH], F32, tag="R_p")
                            nc.tensor.matmul(
                                R_p,
                                lhsT=r(ones_row),
                                rhs=r(rS1r[0:1, hi4, sub * NCH:(sub + 1) * NCH]),
                                start=True, stop=True,
                            )
                            # P1 = expA * (1/S1), in place over expA
                            sl = expA[:, hi4, sub * NCH:(sub + 1) * NCH]
                            nc.vector.tensor_mul(r(sl), r(sl), R_p)
                    nc.scalar.activation(exp2, expA, EXP, scale=SMOOTH)
                    # fold 8 j-blocks of 128 into Pfold for these 4 heads
                    hlo = hb * 8 + qb * 4
                    e2v = exp2.rearrange("p h (j q) -> p h j q", q=LQ)
                    for j in range(SUB * NCH // LQ):
                        if scp == 0 and j == 0:
                            nc.vector.tensor_copy(Pfold[:, hlo:hlo + 4, :],
                                                  e2v[:, :, j, :])
                        else:
                            nc.vector.tensor_add(Pfold[:, hlo:hlo + 4, :],
                                                 Pfold[:, hlo:hlo + 4, :],
                                                 e2v[:, :, j, :])

        # ---- epilogue ----
        x_p = vp_ps.tile([128, 8], F32, tag="vp_p")
        for h in range(H):
            z2 = smallp.tile([128, 1], F32, tag="z2")
            nc.vector.reduce_sum(z2, Pfold[:, h, :], axis=AX)
            rz2 = smallp.tile([128, 1], F32, tag="rz2")
            nc.vector.reciprocal(rz2, z2)
            pfr = smallp.tile([128, LQ], BF16, tag="pfr")
            nc.vector.tensor_copy(pfr, Pfold[:, h, :])
            rz2r = smallp.tile([128, 1], BF16, tag="rz2r")
            nc.vector.tensor_copy(rz2r, rz2)
            w_p = vp_ps.tile([128, 1], F32, tag="vp_p")
            nc.tensor.matmul(w_p, lhsT=pfr, rhs=rz2r,
                             start=True, stop=True)
            wT = smallp.tile([128, 1], BF16, tag="wT")
            nc.vector.tensor_copy(wT, w_p)
            po = 64 * (h % 2)
            nc.tensor.matmul(
                x_p[po:po + 64, h // 2:h // 2 + 1],
                lhsT=iN_sb[:, h, :],
                rhs=wT,
                start=True, stop=True, skip_group_check=True,
            )
        x_sb = singles.tile([128, 8], F32, tag="x_sb")
        nc.vector.tensor_copy(r(x_sb), r(x_p))
        out_sb = singles.tile([1, D], F32, tag="out_sb")
        for nb in range(2):
            # Wo column-half, reusing the vT stream slots
            Wo_sb = vstream.tile([128, KD, NCH], F32, tag="vT")
            nc.sync.dma_start(
                out=r(Wo_sb),
                in_=r(Wo_d[:, nb * NCH:(nb + 1) * NCH].rearrange(
                    "(k p) c -> p k c", p=128)),
            )
            o_p = a_ps.tile([1, NCH], F32, tag="a_p")
            for j in range(KD):
                nc.tensor.matmul(
                    o_p,
                    lhsT=r(x_sb[:, j:j + 1]),
                    rhs=r(Wo_sb[:, j, :]),
                    start=(j == 0), stop=False,
                )
            nc.tensor.matmul(
                o_p,
                lhsT=r(ones_row[0:1, 0:1]),
                rhs=r(bo_sb[:, nb * NCH:(nb + 1) * NCH]),
                start=False, stop=True,
            )
            nc.vector.tensor_copy(out_sb[:, nb * NCH:(nb + 1) * NCH], o_p)
        nc.sync.dma_start(out=out_d, in_=out_sb)

    nc.compile()
    return nc


def make_in_maps(v, i, Wv, bv, Wo, bo):
    """Shard + lay out inputs per core (core c = batch c)."""
    v = np.ascontiguousarray(np.asarray(v, np.float32))
    i = np.ascontiguousarray(np.asarray(i, np.float32))
    Wv = np.ascontiguousarray(np.asarray(Wv, np.float32))
    Wo = np.ascontiguousarray(np.asarray(Wo, np.float32))
    bv = np.asarray(bv, np.float32)
    bo = np.ascontiguousarray(np.asarray(bo, np.float32)).reshape(1, D)
    in_maps = []
    for b in range(B):
        hv = i[b * H:(b + 1) * H]                      # (16, 128, 64)
        iT = np.zeros((128, 8, 128), np.float32)
        for h in range(H):
            iT[64 * (h % 2):64 * (h % 2) + 64, h // 2, :] = hv[h].T
        import ml_dtypes
        iN = np.ascontiguousarray(np.transpose(hv, (1, 0, 2))).astype(
            ml_dtypes.bfloat16)                                  # (128, 16, 64)
        # C shifts softmax1 logits (exact for softmax; keeps exp/ln in a
        # well-conditioned range for the ACT splines)
        cbm = (np.einsum("hqd,hd->qh", hv, bv.reshape(H, DK))
               - 28.0).astype(np.float32)                       # (128, 16)
        ind = np.zeros((128, 8, 8), np.float32)
        for hi in range(8):
            ind[:, hi, hi] = 1.0
        in_maps.append({
            "ones": np.ones((1, 128), np.float32),
            "ind": ind,
            "vT": np.ascontiguousarray(v[b].T),
            "iT": iT,
            "iN": iN,
            "cb": np.ascontiguousarray(cbm),
            "Wv": Wv,
            "Wo": Wo,
            "bo": bo,
        })
    return in_maps


_NC_CACHE = None


def kernel(q, k, v, i, Wq, bq, Wk, bk, Wv, bv, Wo, bo):
    global _NC_CACHE
    if _NC_CACHE is None:
        _NC_CACHE = build_program()
    nc = _NC_CACHE
    in_maps = make_in_maps(v, i, Wv, bv, Wo, bo)
    res = run_bass_kernel_spmd(nc, in_maps, list(range(8)))
    rows = [res.results[c]["out"].reshape(1, D) for c in range(B)]
    return np.stack(rows, axis=0).astype(np.float32)  # (8, 1, 1024)


if __name__ == "__main__":
    build_program()
    print("compiled OK")


# revision 14
# speedup vs baseline: 1.0588x; 1.0588x over previous
"""Trainium2 Bass kernel for nn_MultiHeadAttention_47330539602717.

Math (per batch b, head h; q/k projections are dead code in the reference):
    vpT   = Wv^T @ v_b^T                        (1024, 4096)  [no bias]
    A^T_h = i_h @ vpT_h + (i_h @ bv_h)[:,None]  (128 q, 4096 s)
    P1    = exp(A^T) / colsum(exp(A^T))         softmax over q (partition dim)
    exp2  = exp(9 * P1)
    Pfold[l, qr] = sum_j exp2[l, qr + 128 j]    (torch raw .view fold)
    w[q]  = sum_l Pfold[l, q] / Z2[l],  Z2[l] = sum_qr Pfold[l, qr]
    x_h   = w @ i_h                             (64,)
    out_b = concat_h(x_h) @ Wo + bo             (1, 1024)

Sharding: data-parallel over batch. Core c handles batch b=c (all 16 heads).
Engine plan per core: PE does the big GEMM (f32r full-rate), per-head A^T,
S1 column-sums (indicator matmuls), and S1-replication; ACT does exp1 (with
the bv bias folded per-partition), exp2, and 1/S1 via Ln+Exp(-x); DVE does
the softmax1 divide-multiply, the j-fold adds, and PSUM->SBUF moves.
"""

import sys

import numpy as np

sys.path.insert(0, "/opt/trn_rl_repo")

from contextlib import ExitStack

import concourse.bacc as bacc
import concourse.tile as tile
from concourse import mybir
from concourse.bass_utils import run_bass_kernel_spmd

F32 = mybir.dt.float32
F32R = mybir.dt.float32r
BF16 = mybir.dt.bfloat16
EXP = mybir.ActivationFunctionType.Exp
LN = mybir.ActivationFunctionType.Ln
AX = mybir.AxisListType.X

B, LQ, S, D, H = 8, 128, 4096, 1024, 16
DK = D // H          # 64
KD = 8               # k blocks of 128 over D
SCP = 4              # outer s chunks (1024 cols each)
SUB = 2              # 512-col substeps per chunk
NCH = 512
SMOOTH = 9.0


def r(ap):
    return ap.bitcast(F32R)


def build_program():
    nc = bacc.Bacc("TRN2", target_bir_lowering=False, debug=False,
                   num_devices=8)

    vT_d = nc.dram_tensor("vT", [D, S], F32, kind="ExternalInput").ap()
    iT_d = nc.dram_tensor("iT", [128, 8, 128], F32, kind="ExternalInput").ap()
    iN_d = nc.dram_tensor("iN", [128, H, DK], BF16, kind="ExternalInput").ap()
    cb_d = nc.dram_tensor("cb", [128, H], F32, kind="ExternalInput").ap()
    Wv_d = nc.dram_tensor("Wv", [D, D], F32, kind="ExternalInput").ap()
    Wo_d = nc.dram_tensor("Wo", [D, D], F32, kind="ExternalInput").ap()
    bo_d = nc.dram_tensor("bo", [1, D], F32, kind="ExternalInput").ap()
    ones_d = nc.dram_tensor("ones", [1, 128], F32, kind="ExternalInput").ap()
    ind_d = nc.dram_tensor("ind", [128, 8, 8], F32, kind="ExternalInput").ap()
    out_d = nc.dram_tensor("out", [1, D], F32, kind="ExternalOutput").ap()

    with tile.TileContext(nc) as tc, ExitStack() as ctx:
        singles = ctx.enter_context(tc.tile_pool(name="singles", bufs=1))
        vstream = ctx.enter_context(tc.tile_pool(name="vstream", bufs=2))
        vppool = ctx.enter_context(tc.tile_pool(name="vppool", bufs=3))
        expap = ctx.enter_context(tc.tile_pool(name="expap", bufs=2))
        e2pool = ctx.enter_context(tc.tile_pool(name="e2pool", bufs=1))
        smallp = ctx.enter_context(tc.tile_pool(name="smalls", bufs=2))
        rowp = ctx.enter_context(tc.tile_pool(name="rowp", bufs=2))
        vp_ps = ctx.enter_context(tc.tile_pool(name="vp_ps", bufs=2, space="PSUM"))
        a_ps = ctx.enter_context(tc.tile_pool(name="a_ps", bufs=2, space="PSUM"))
        R_ps = ctx.enter_context(tc.tile_pool(name="R_ps", bufs=2, space="PSUM"))
        s1_ps = ctx.enter_context(tc.tile_pool(name="s1_ps", bufs=1, space="PSUM"))

        # ---- constants / weights ----
        Wv_sb = singles.tile([128, KD, D], F32)       # ktile k at [:, k, :]
        nc.sync.dma_start(out=r(Wv_sb), in_=r(Wv_d.rearrange("(k p) c -> p k c", p=128)))
        iT_sb = singles.tile([128, 8, 128], F32)      # head h: [64*(h%2):+64, h//2, :]
        nc.sync.dma_start(out=r(iT_sb), in_=r(iT_d))
        iN_sb = singles.tile([128, H, DK], BF16)      # i_h as (q, d) at [:, h, :]
        nc.sync.dma_start(out=iN_sb, in_=iN_d)
        cb_sb = singles.tile([128, H], F32)
        nc.sync.dma_start(out=cb_sb, in_=cb_d)
        bo_sb = singles.tile([1, D], F32)
        nc.sync.dma_start(out=r(bo_sb), in_=r(bo_d))
        ones_row = singles.tile([1, 128], F32)        # lhsT for replicate (K=1,M=128)
        nc.sync.dma_start(out=r(ones_row), in_=r(ones_d))
        ind8 = singles.tile([128, 8, 8], F32)         # indicator lhsT per head-in-half
        nc.sync.dma_start(out=r(ind8), in_=r(ind_d))
        Pfold = singles.tile([128, H, LQ], F32)       # per-head fold accumulators

        # ---- main loop over 1024-col chunks ----
        for scp in range(SCP):
            vpT = []  # two (128, KD, 512) SBUF tiles, one per 512 substep
            for sub in range(SUB):
                sidx = scp * SUB + sub
                vT_sb = vstream.tile([128, KD, NCH], F32, tag="vT")
                nc.sync.dma_start(
                    out=r(vT_sb),
                    in_=r(vT_d[:, sidx * NCH:(sidx + 1) * NCH].rearrange(
                        "(k p) s -> p k s", p=128)),
                )
                vp_sb = vppool.tile([128, KD, NCH], F32, tag="vp")
                for cb in range(KD):
                    vp_p = vp_ps.tile([128, NCH], F32, tag="vp_p")
                    for k in range(KD):
                        nc.tensor.matmul(
                            vp_p,
                            lhsT=r(Wv_sb[:, k, cb * 128:(cb + 1) * 128]),
                            rhs=r(vT_sb[:, k, :]),
                            start=(k == 0), stop=(k == KD - 1),
                        )
                    nc.vector.tensor_copy(r(vp_sb[:, cb, :]), r(vp_p))
                vpT.append(vp_sb)

            for hb in range(2):
                s1_p = s1_ps.tile([8, SUB, NCH], F32, tag="s1")
                expq = []  # two (128, 4, 1024) expA quarter tiles
                for qb in range(2):
                    expA = expap.tile([128, 4, SUB * NCH], F32, tag="expa")
                    expq.append(expA)
                    for hi4 in range(4):
                        h = hb * 8 + qb * 4 + hi4
                        hi8 = qb * 4 + hi4
                        po = 64 * (h % 2)
                        for sub in range(SUB):
                            a_p = a_ps.tile([128, NCH], F32, tag="a_p")
                            nc.tensor.matmul(
                                a_p,
                                lhsT=r(iT_sb[po:po + 64, h // 2, :]),
                                rhs=r(vpT[sub][po:po + 64, h // 2, :]),
                                start=True, stop=True,
                            )
                            nc.scalar.activation(
                                r(expA[:, hi4, sub * NCH:(sub + 1) * NCH]),
                                a_p, EXP, bias=cb_sb[:, h:h + 1])
                        for sub in range(SUB):
                            nc.tensor.matmul(
                                s1_p[:, sub, :],
                                lhsT=r(ind8[:, hi8, :]),
                                rhs=r(expA[:, hi4, sub * NCH:(sub + 1) * NCH]),
                                start=(hi8 == 0), stop=(hi8 == 7),
                            )
                # 1/S1 for the half: spread the (8,1024) rows over all 128
                # partitions via DMA, reciprocal on DVE, then flatten to a
                # partition-0 row for the PE replicate matmuls
                s1_sb = smallp.tile([8, SUB * NCH], F32, tag="s1sb")
                nc.scalar.activation(s1_sb, s1_p,
                                     mybir.ActivationFunctionType.Copy)
                s1_sq = smallp.tile([128, 64], F32, tag="s1sq")
                nc.sync.dma_start(out=s1_sq, in_=s1_sb)
                rsq = smallp.tile([128, 64], F32, tag="rsq")
                nc.vector.reciprocal(rsq, s1_sq)
                for qb in range(2):
                    expA = expq[qb]
                    # PE matmul operands must share base partition 0: flatten
                    # 1/S1 rows (2 heads at a time) into partition-0 row tiles
                    rows2 = []
                    for pr in range(2):
                        rr = rowp.tile([1, 2, SUB * NCH], F32, tag="rS1r")
                        nc.sync.dma_start(
                            out=r(rr),
                            in_=r(rsq[qb * 64 + pr * 32:qb * 64 + (pr + 1) * 32, :]))
                        rows2.append(rr)
                    exp2 = e2pool.tile([128, 4, SUB * NCH], BF16, tag="exp2")
                    for hi4 in range(4):
                        h = hb * 8 + qb * 4 + hi4
                        hi8 = qb * 4 + hi4
                        for sub in range(SUB):
                            R_p = R_ps.tile([128, NCH], F32, tag="R_p")
                            nc.tensor.matmul(
                                R_p,
                                lhsT=r(ones_row),
                                rhs=r(rows2[hi4 // 2][0:1, hi4 % 2, sub * NCH:(sub + 1) * NCH]),
                                start=True, stop=True,
                            )
                            # P1 = expA * (1/S1), in place over expA
                            sl = expA[:, hi4, sub * NCH:(sub + 1) * NCH]
                            nc.vector.tensor_mul(r(sl), r(sl), R_p)
                    nc.scalar.activation(exp2, expA, EXP, scale=SMOOTH)
                    # fold 8 j-blocks of 128 into Pfold for these 4 heads
                    hlo = hb * 8 + qb * 4
                    e2v = exp2.rearrange("p h (j q) -> p h j q", q=LQ)
                    for j in range(SUB * NCH // LQ):
                        if scp == 0 and j == 0:
                            nc.vector.tensor_copy(Pfold[:, hlo:hlo + 4, :],
                                                  e2v[:, :, j, :])
                        else:
                            nc.vector.tensor_add(Pfold[:, hlo:hlo + 4, :],
                                                 Pfold[:, hlo:hlo + 4, :],
                                                 e2v[:, :, j, :])

        # ---- epilogue ----
        x_p = vp_ps.tile([128, 8], F32, tag="vp_p")
        for h in range(H):
            z2 = smallp.tile([128, 1], F32, tag="z2")
            nc.vector.reduce_sum(z2, Pfold[:, h, :], axis=AX)
            rz2 = smallp.tile([128, 1], F32, tag="rz2")
            nc.vector.reciprocal(rz2, z2)
            pfr = smallp.tile([128, LQ], BF16, tag="pfr")
            nc.vector.tensor_copy(pfr, Pfold[:, h, :])
            rz2r = smallp.tile([128, 1], BF16, tag="rz2r")
            nc.vector.tensor_copy(rz2r, rz2)
            w_p = vp_ps.tile([128, 1], F32, tag="vp_p")
            nc.tensor.matmul(w_p, lhsT=pfr, rhs=rz2r,
                             start=True, stop=True)
            wT = smallp.tile([128, 1], BF16, tag="wT")
            nc.vector.tensor_copy(wT, w_p)
            po = 64 * (h % 2)
            nc.tensor.matmul(
                x_p[po:po + 64, h // 2:h // 2 + 1],
                lhsT=iN_sb[:, h, :],
                rhs=wT,
                start=True, stop=True, skip_group_check=True,
            )
        x_sb = singles.tile([128, 8], F32, tag="x_sb")
        nc.vector.tensor_copy(r(x_sb), r(x_p))
        out_sb = singles.tile([1, D], F32, tag="out_sb")
        for nb in range(2):
            # Wo column-half, reusing the vT stream slots
            Wo_sb = vstream.tile([128, KD, NCH], F32, tag="vT")
            nc.sync.dma_start(
                out=r(Wo_sb),
                in_=r(Wo_d[:, nb * NCH:(nb + 1) * NCH].rearrange(
                    "(k p) c -> p k c", p=128)),
            )
            o_p = a_ps.tile([1, NCH], F32, tag="a_p")
            for j in range(KD):
                nc.tensor.matmul(
                    o_p,
                    lhsT=r(x_sb[:, j:j + 1]),
                    rhs=r(Wo_sb[:, j, :]),
                    start=(j == 0), stop=False,
                )
            nc.tensor.matmul(
                o_p,
                lhsT=r(ones_row[0:1, 0:1]),
                rhs=r(bo_sb[:, nb * NCH:(nb + 1) * NCH]),
                start=False, stop=True,
            )
            nc.vector.tensor_copy(out_sb[:, nb * NCH:(nb + 1) * NCH], o_p)
        nc.sync.dma_start(out=out_d, in_=out_sb)

    nc.compile()
    return nc


def make_in_maps(v, i, Wv, bv, Wo, bo):
    """Shard + lay out inputs per core (core c = batch c)."""
    v = np.ascontiguousarray(np.asarray(v, np.float32))
    i = np.ascontiguousarray(np.asarray(i, np.float32))
    Wv = np.ascontiguousarray(np.asarray(Wv, np.float32))
    Wo = np.ascontiguousarray(np.asarray(Wo, np.float32))
    bv = np.asarray(bv, np.float32)
    bo = np.ascontiguousarray(np.asarray(bo, np.float32)).reshape(1, D)
    in_maps = []
    for b in range(B):
        hv = i[b * H:(b + 1) * H]                      # (16, 128, 64)
        iT = np.zeros((128, 8, 128), np.float32)
        for h in range(H):
            iT[64 * (h % 2):64 * (h % 2) + 64, h // 2, :] = hv[h].T
        import ml_dtypes
        iN = np.ascontiguousarray(np.transpose(hv, (1, 0, 2))).astype(
            ml_dtypes.bfloat16)                                  # (128, 16, 64)
        # C shifts softmax1 logits (exact for softmax; keeps exp/ln in a
        # well-conditioned range for the ACT splines)
        cbm = (np.einsum("hqd,hd->qh", hv, bv.reshape(H, DK))
               - 28.0).astype(np.float32)                       # (128, 16)
        ind = np.zeros((128, 8, 8), np.float32)
        for hi in range(8):
            ind[:, hi, hi] = 1.0
        in_maps.append({
            "ones": np.ones((1, 128), np.float32),
            "ind": ind,
            "vT": np.ascontiguousarray(v[b].T),
            "iT": iT,
            "iN": iN,
            "cb": np.ascontiguousarray(cbm),
            "Wv": Wv,
            "Wo": Wo,
            "bo": bo,
        })
    return in_maps


_NC_CACHE = None


def kernel(q, k, v, i, Wq, bq, Wk, bk, Wv, bv, Wo, bo):
    global _NC_CACHE
    if _NC_CACHE is None:
        _NC_CACHE = build_program()
    nc = _NC_CACHE
    in_maps = make_in_maps(v, i, Wv, bv, Wo, bo)
    res = run_bass_kernel_spmd(nc, in_maps, list(range(8)))
    rows = [res.results[c]["out"].reshape(1, D) for c in range(B)]
    return np.stack(rows, axis=0).astype(np.float32)  # (8, 1, 1024)


if __name__ == "__main__":
    build_program()
    print("compiled OK")


# revision 16
# speedup vs baseline: 1.1139x; 1.0520x over previous
"""Trainium2 Bass kernel for nn_MultiHeadAttention_47330539602717.

Math (per batch b, head h; q/k projections are dead code in the reference):
    vpT   = Wv^T @ v_b^T                        (1024, 4096)  [no bias]
    A^T_h = i_h @ vpT_h + (i_h @ bv_h)[:,None]  (128 q, 4096 s)
    P1    = exp(A^T) / colsum(exp(A^T))         softmax over q (partition dim)
    exp2  = exp(9 * P1)
    Pfold[l, qr] = sum_j exp2[l, qr + 128 j]    (torch raw .view fold)
    w[q]  = sum_l Pfold[l, q] / Z2[l],  Z2[l] = sum_qr Pfold[l, qr]
    x_h   = w @ i_h                             (64,)
    out_b = concat_h(x_h) @ Wo + bo             (1, 1024)

Sharding: data-parallel over batch. Core c handles batch b=c (all 16 heads).
Engine plan per core: PE does the big GEMM (f32r full-rate), per-head A^T,
S1 column-sums (indicator matmuls), and S1-replication; ACT does exp1 (with
the bv bias folded per-partition), exp2, and 1/S1 via Ln+Exp(-x); DVE does
the softmax1 divide-multiply, the j-fold adds, and PSUM->SBUF moves.
"""

import sys

import numpy as np

sys.path.insert(0, "/opt/trn_rl_repo")

from contextlib import ExitStack

import concourse.bacc as bacc
import concourse.tile as tile
from concourse import mybir
from concourse.bass_utils import run_bass_kernel_spmd

F32 = mybir.dt.float32
F32R = mybir.dt.float32r
BF16 = mybir.dt.bfloat16
EXP = mybir.ActivationFunctionType.Exp
LN = mybir.ActivationFunctionType.Ln
AX = mybir.AxisListType.X

B, LQ, S, D, H = 8, 128, 4096, 1024, 16
DK = D // H          # 64
KD = 8               # k blocks of 128 over D
SCP = 4              # outer s chunks (1024 cols each)
SUB = 2              # 512-col substeps per chunk
NCH = 512
SMOOTH = 9.0


def r(ap):
    return ap.bitcast(F32R)


def build_program():
    nc = bacc.Bacc("TRN2", target_bir_lowering=False, debug=False,
                   num_devices=8)

    vT_d = nc.dram_tensor("vT", [D, S], F32, kind="ExternalInput").ap()
    iT_d = nc.dram_tensor("iT", [128, 8, 128], F32, kind="ExternalInput").ap()
    iN_d = nc.dram_tensor("iN", [128, H, DK], BF16, kind="ExternalInput").ap()
    cb_d = nc.dram_tensor("cb", [128, H], F32, kind="ExternalInput").ap()
    Wv_d = nc.dram_tensor("Wv", [D, D], F32, kind="ExternalInput").ap()
    Wo_d = nc.dram_tensor("Wo", [D, D], F32, kind="ExternalInput").ap()
    bo_d = nc.dram_tensor("bo", [1, D], F32, kind="ExternalInput").ap()
    ones_d = nc.dram_tensor("ones", [1, 128], F32, kind="ExternalInput").ap()
    ind_d = nc.dram_tensor("ind", [128, 8, 8], F32, kind="ExternalInput").ap()
    out_d = nc.dram_tensor("out", [1, D], F32, kind="ExternalOutput").ap()

    with tile.TileContext(nc) as tc, ExitStack() as ctx:
        singles = ctx.enter_context(tc.tile_pool(name="singles", bufs=1))
        vstream = ctx.enter_context(tc.tile_pool(name="vstream", bufs=2))
        vppool = ctx.enter_context(tc.tile_pool(name="vppool", bufs=3))
        expap = ctx.enter_context(tc.tile_pool(name="expap", bufs=2))
        e2pool = ctx.enter_context(tc.tile_pool(name="e2pool", bufs=1))
        smallp = ctx.enter_context(tc.tile_pool(name="smalls", bufs=2))
        rowp = ctx.enter_context(tc.tile_pool(name="rowp", bufs=2))
        vp_ps = ctx.enter_context(tc.tile_pool(name="vp_ps", bufs=2, space="PSUM"))
        aR_ps = ctx.enter_context(tc.tile_pool(name="aR_ps", bufs=2, space="PSUM"))
        s1_ps = ctx.enter_context(tc.tile_pool(name="s1_ps", bufs=1, space="PSUM"))

        # ---- constants / weights ----
        Wv_sb = singles.tile([128, KD, D], F32)       # ktile k at [:, k, :]
        nc.sync.dma_start(out=r(Wv_sb), in_=r(Wv_d.rearrange("(k p) c -> p k c", p=128)))
        iT_sb = singles.tile([128, 8, 128], F32)      # head h: [64*(h%2):+64, h//2, :]
        nc.sync.dma_start(out=r(iT_sb), in_=r(iT_d))
        iN_sb = singles.tile([128, H, DK], BF16)      # i_h as (q, d) at [:, h, :]
        nc.sync.dma_start(out=iN_sb, in_=iN_d)
        cb_sb = singles.tile([128, H], F32)
        nc.sync.dma_start(out=cb_sb, in_=cb_d)
        bo_sb = singles.tile([1, D], F32)
        nc.sync.dma_start(out=r(bo_sb), in_=r(bo_d))
        ones_row = singles.tile([1, 128], F32)        # lhsT for replicate (K=1,M=128)
        nc.sync.dma_start(out=r(ones_row), in_=r(ones_d))
        ind8 = singles.tile([128, 8, 8], F32)         # indicator lhsT per head-in-half
        nc.sync.dma_start(out=r(ind8), in_=r(ind_d))
        Pfold = singles.tile([128, H, LQ], F32)       # per-head fold accumulators

        # ---- main loop over 1024-col chunks ----
        for scp in range(SCP):
            vpT = []  # two (128, KD, 512) SBUF tiles, one per 512 substep
            for sub in range(SUB):
                sidx = scp * SUB + sub
                vT_sb = vstream.tile([128, KD, NCH], F32, tag="vT")
                nc.sync.dma_start(
                    out=r(vT_sb),
                    in_=r(vT_d[:, sidx * NCH:(sidx + 1) * NCH].rearrange(
                        "(k p) s -> p k s", p=128)),
                )
                vp_sb = vppool.tile([128, KD, NCH], F32, tag="vp")
                for cb in range(KD):
                    vp_p = vp_ps.tile([128, NCH], F32, tag="vp_p")
                    for k in range(KD):
                        nc.tensor.matmul(
                            vp_p,
                            lhsT=r(Wv_sb[:, k, cb * 128:(cb + 1) * 128]),
                            rhs=r(vT_sb[:, k, :]),
                            start=(k == 0), stop=(k == KD - 1),
                        )
                    nc.vector.tensor_copy(r(vp_sb[:, cb, :]), r(vp_p))
                vpT.append(vp_sb)

            for hb in range(2):
                s1_p = s1_ps.tile([8, SUB, NCH], F32, tag="s1")
                expq = []  # two (128, 4, 1024) expA quarter tiles
                for qb in range(2):
                    expA = expap.tile([128, 4, SUB * NCH], F32, tag="expa")
                    expq.append(expA)
                    for hi4 in range(4):
                        h = hb * 8 + qb * 4 + hi4
                        hi8 = qb * 4 + hi4
                        po = 64 * (h % 2)
                        a_p = aR_ps.tile([128, SUB, NCH], F32, tag="aR")
                        for sub in range(SUB):
                            nc.tensor.matmul(
                                a_p[:, sub, :],
                                lhsT=r(iT_sb[po:po + 64, h // 2, :]),
                                rhs=r(vpT[sub][po:po + 64, h // 2, :]),
                                start=True, stop=True,
                            )
                        nc.scalar.activation(r(expA[:, hi4, :]), a_p, EXP,
                                             bias=cb_sb[:, h:h + 1])
                        for sub in range(SUB):
                            nc.tensor.matmul(
                                s1_p[:, sub, :],
                                lhsT=r(ind8[:, hi8, :]),
                                rhs=r(expA[:, hi4, sub * NCH:(sub + 1) * NCH]),
                                start=(hi8 == 0), stop=(hi8 == 7),
                            )
                # 1/S1 for the half: copy to SBUF, spread rows over all 128
                # partitions via DMA, reciprocal on DVE
                s1_sb = smallp.tile([8, SUB * NCH], F32, tag="s1sb")
                nc.scalar.activation(s1_sb, s1_p,
                                     mybir.ActivationFunctionType.Copy)
                s1_sq = smallp.tile([128, 64], F32, tag="s1sq")
                nc.sync.dma_start(out=s1_sq, in_=s1_sb)
                rsq = smallp.tile([128, 64], F32, tag="rsq")
                nc.vector.reciprocal(rsq, s1_sq)
                for qb in range(2):
                    expA = expq[qb]
                    # PE matmul operands must share base partition 0: flatten
                    # 1/S1 rows (2 heads at a time) into partition-0 row tiles
                    rows2 = []
                    for pr in range(2):
                        rr = rowp.tile([1, 2, SUB * NCH], F32, tag="rS1r")
                        nc.sync.dma_start(
                            out=r(rr),
                            in_=r(rsq[qb * 64 + pr * 32:qb * 64 + (pr + 1) * 32, :]))
                        rows2.append(rr)
                    exp2 = e2pool.tile([128, 4, SUB * NCH], BF16, tag="exp2")
                    for hi4 in range(4):
                        h = hb * 8 + qb * 4 + hi4
                        hi8 = qb * 4 + hi4
                        R_p = aR_ps.tile([128, SUB, NCH], F32, tag="aR")
                        for sub in range(SUB):
                            nc.tensor.matmul(
                                R_p[:, sub, :],
                                lhsT=r(ones_row),
                                rhs=r(rows2[hi4 // 2][0:1, hi4 % 2, sub * NCH:(sub + 1) * NCH]),
                                start=True, stop=True,
                            )
                        # P1 = expA * (1/S1), in place over expA
                        nc.vector.tensor_mul(r(expA[:, hi4, :]), r(expA[:, hi4, :]), R_p)
                    nc.scalar.activation(exp2, expA, EXP, scale=SMOOTH)
                    # fold 8 j-blocks of 128 into Pfold for these 4 heads
                    hlo = hb * 8 + qb * 4
                    e2v = exp2.rearrange("p h (j q) -> p h j q", q=LQ)
                    for j in range(SUB * NCH // LQ):
                        if scp == 0 and j == 0:
                            nc.vector.tensor_copy(Pfold[:, hlo:hlo + 4, :],
                                                  e2v[:, :, j, :])
                        else:
                            nc.vector.tensor_add(Pfold[:, hlo:hlo + 4, :],
                                                 Pfold[:, hlo:hlo + 4, :],
                                                 e2v[:, :, j, :])

        # ---- epilogue ----
        x_p = vp_ps.tile([128, 8], F32, tag="vp_p")
        for h in range(H):
            z2 = smallp.tile([128, 1], F32, tag="z2")
            nc.vector.reduce_sum(z2, Pfold[:, h, :], axis=AX)
            rz2 = smallp.tile([128, 1], F32, tag="rz2")
            nc.vector.reciprocal(rz2, z2)
            pfr = smallp.tile([128, LQ], BF16, tag="pfr")
            nc.vector.tensor_copy(pfr, Pfold[:, h, :])
            rz2r = smallp.tile([128, 1], BF16, tag="rz2r")
            nc.vector.tensor_copy(rz2r, rz2)
            w_p = vp_ps.tile([128, 1], F32, tag="vp_p")
            nc.tensor.matmul(w_p, lhsT=pfr, rhs=rz2r,
                             start=True, stop=True)
            wT = smallp.tile([128, 1], BF16, tag="wT")
            nc.vector.tensor_copy(wT, w_p)
            po = 64 * (h % 2)
            nc.tensor.matmul(
                x_p[po:po + 64, h // 2:h // 2 + 1],
                lhsT=iN_sb[:, h, :],
                rhs=wT,
                start=True, stop=True, skip_group_check=True,
            )
        x_sb = singles.tile([128, 8], F32, tag="x_sb")
        nc.vector.tensor_copy(r(x_sb), r(x_p))
        out_sb = singles.tile([1, D], F32, tag="out_sb")
        for nb in range(2):
            # Wo column-half, reusing the vT stream slots
            Wo_sb = vstream.tile([128, KD, NCH], F32, tag="vT")
            nc.sync.dma_start(
                out=r(Wo_sb),
                in_=r(Wo_d[:, nb * NCH:(nb + 1) * NCH].rearrange(
                    "(k p) c -> p k c", p=128)),
            )
            o_p = aR_ps.tile([1, NCH], F32, tag="aR")
            for j in range(KD):
                nc.tensor.matmul(
                    o_p,
                    lhsT=r(x_sb[:, j:j + 1]),
                    rhs=r(Wo_sb[:, j, :]),
                    start=(j == 0), stop=False,
                )
            nc.tensor.matmul(
                o_p,
                lhsT=r(ones_row[0:1, 0:1]),
                rhs=r(bo_sb[:, nb * NCH:(nb + 1) * NCH]),
                start=False, stop=True,
            )
            nc.vector.tensor_copy(out_sb[:, nb * NCH:(nb + 1) * NCH], o_p)
        nc.sync.dma_start(out=out_d, in_=out_sb)

    nc.compile()
    return nc


def make_in_maps(v, i, Wv, bv, Wo, bo):
    """Shard + lay out inputs per core (core c = batch c)."""
    v = np.ascontiguousarray(np.asarray(v, np.float32))
    i = np.ascontiguousarray(np.asarray(i, np.float32))
    Wv = np.ascontiguousarray(np.asarray(Wv, np.float32))
    Wo = np.ascontiguousarray(np.asarray(Wo, np.float32))
    bv = np.asarray(bv, np.float32)
    bo = np.ascontiguousarray(np.asarray(bo, np.float32)).reshape(1, D)
    in_maps = []
    for b in range(B):
        hv = i[b * H:(b + 1) * H]                      # (16, 128, 64)
        iT = np.zeros((128, 8, 128), np.float32)
        for h in range(H):
            iT[64 * (h % 2):64 * (h % 2) + 64, h // 2, :] = hv[h].T
        import ml_dtypes
        iN = np.ascontiguousarray(np.transpose(hv, (1, 0, 2))).astype(
            ml_dtypes.bfloat16)                                  # (128, 16, 64)
        # C shifts softmax1 logits (exact for softmax; keeps exp/ln in a
        # well-conditioned range for the ACT splines)
        cbm = (np.einsum("hqd,hd->qh", hv, bv.reshape(H, DK))
               - 28.0).astype(np.float32)                       # (128, 16)
        ind = np.zeros((128, 8, 8), np.float32)
        for hi in range(8):
            ind[:, hi, hi] = 1.0
        in_maps.append({
            "ones": np.ones((1, 128), np.float32),
            "ind": ind,
            "vT": np.ascontiguousarray(v[b].T),
            "iT": iT,
            "iN": iN,
            "cb": np.ascontiguousarray(cbm),
            "Wv": Wv,
            "Wo": Wo,
            "bo": bo,
        })
    return in_maps


_NC_CACHE = None


def kernel(q, k, v, i, Wq, bq, Wk, bk, Wv, bv, Wo, bo):
    global _NC_CACHE
    if _NC_CACHE is None:
        _NC_CACHE = build_program()
    nc = _NC_CACHE
    in_maps = make_in_maps(v, i, Wv, bv, Wo, bo)
    res = run_bass_kernel_spmd(nc, in_maps, list(range(8)))
    rows = [res.results[c]["out"].reshape(1, D) for c in range(B)]
    return np.stack(rows, axis=0).astype(np.float32)  # (8, 1, 1024)


if __name__ == "__main__":
    build_program()
    print("compiled OK")
